# revision 5
# baseline (speedup 1.0000x reference)
"""Trainium2 Bass kernel: 4-layer causal transformer (B=2, T=2048, D=512, H=8, FF=2048).

Sharding: sequence-parallel across 8 NeuronCores — core c owns the contiguous
512-token block (c%4) of batch (c//4) for all dense/LN work. Attention is
head-parallel: one 8-core AllToAll redistributes per-head Q/K/V so core j
computes full causal attention for head j of BOTH batches (identical static
program on every core); a second AllToAll returns attention outputs to the
token owners. Dense matmuls run fp32r (full PE rate at N>=512), attention
matmuls run bf16; all accumulation is fp32 in PSUM. The reference ALiBi bias
is identically zero on unmasked positions (triu zeroes j<i, diag is 0), so
attention is plain causal softmax; logits are tiny (|logit| < 2), so softmax
runs without max subtraction. LayerNorm is computed feature-major using
ones-matmul partition reductions and K=1 broadcast matmuls; rstd comes from
exp(-0.5*ln(var+eps)) to stay inside the ACT exp/ln table set.
"""

import math
import os
import sys

for _p in ("/opt/trn_rl_repo", "/root/.axon_site/_ro/trn_rl_repo"):
    if os.path.isdir(_p) and _p not in sys.path:
        sys.path.insert(0, _p)

import numpy as np
import ml_dtypes

import concourse.bass as bass
import concourse.bacc as bacc
import concourse.mybir as mybir
import concourse.tile as tile
from concourse.bass_utils import run_bass_kernel_spmd

F32 = mybir.dt.float32
F32R = mybir.dt.float32r
BF16 = mybir.dt.bfloat16
AF = mybir.ActivationFunctionType
OP = mybir.AluOpType

B, T, IN_DIM, D, H, L, OUT = 2, 2048, 128, 512, 8, 4, 128
HD = D // H          # 64
FF = 4 * D           # 2048
NCORES = 8
TOK = 512            # tokens per core
NSEG = 4             # 512-token query segments per batch
NKC = 16             # 128-token key chunks per batch
DT = D // 128        # 4 feature tiles of the residual stream
FFT = FF // 128      # 16

# "f32r": weights/activations fp32, matmuls in fp32r.  "bf16": weights
# host-converted to bf16, dense matmuls in bf16 (half the weight DMA).
MM_MODE = "f32r"
WDT = F32R if MM_MODE == "f32r" else BF16
WNP = np.float32 if MM_MODE == "f32r" else ml_dtypes.bfloat16

EPS = 1e-5
SCALE = 1.0 / math.sqrt(HD)  # 0.125


def _build_program():
    nc = bacc.Bacc("TRN2", target_bir_lowering=False, num_devices=NCORES)

    # ---- per-core external inputs -------------------------------------
    xT = nc.dram_tensor("xT", [IN_DIM, TOK], WDT, kind="ExternalInput")
    peT = nc.dram_tensor("peT", [D, TOK], F32, kind="ExternalInput")
    win = nc.dram_tensor("win", [IN_DIM, D], WDT, kind="ExternalInput")
    binc = nc.dram_tensor("binc", [D, 1], F32, kind="ExternalInput")
    wqkv = nc.dram_tensor("wqkv", [L, D, 3 * D], WDT, kind="ExternalInput")
    bqkvc = nc.dram_tensor("bqkvc", [L, 3 * D, 1], F32, kind="ExternalInput")
    wo = nc.dram_tensor("wo", [L, D, D], WDT, kind="ExternalInput")
    boc = nc.dram_tensor("boc", [L, D, 1], F32, kind="ExternalInput")
    wf1 = nc.dram_tensor("wf1", [L, D, FF], WDT, kind="ExternalInput")
    bf1c = nc.dram_tensor("bf1c", [L, FF, 1], F32, kind="ExternalInput")
    wf2 = nc.dram_tensor("wf2", [L, FF, D], WDT, kind="ExternalInput")
    bf2c = nc.dram_tensor("bf2c", [L, D, 1], F32, kind="ExternalInput")
    ln1g = nc.dram_tensor("ln1g", [L, 1, D], WDT, kind="ExternalInput")
    ln1b = nc.dram_tensor("ln1b", [L, D, 1], F32, kind="ExternalInput")
    ln2g = nc.dram_tensor("ln2g", [L, 1, D], WDT, kind="ExternalInput")
    ln2b = nc.dram_tensor("ln2b", [L, D, 1], F32, kind="ExternalInput")
    lnfg = nc.dram_tensor("lnfg", [1, D], WDT, kind="ExternalInput")
    lnfb = nc.dram_tensor("lnfb", [D, 1], F32, kind="ExternalInput")
    wout = nc.dram_tensor("wout", [D, OUT], WDT, kind="ExternalInput")
    boutr = nc.dram_tensor("boutr", [1, OUT], WDT, kind="ExternalInput")
    trimask_d = nc.dram_tensor("trimask", [128, 128], BF16, kind="ExternalInput")
    out_d = nc.dram_tensor("out", [TOK, OUT], F32, kind="ExternalOutput")

    with tile.TileContext(nc) as tc:
        _emit(nc, tc, locals())
    nc.compile()
    return nc


def _emit(nc, tc, t):
    from contextlib import ExitStack

    ctx = ExitStack()
    with ctx:
        dram = ctx.enter_context(tc.tile_pool(name="dram", bufs=1, space="DRAM"))
        cn = ctx.enter_context(tc.tile_pool(name="const", bufs=1))
        hp = ctx.enter_context(tc.tile_pool(name="hpool", bufs=1))
        yp = ctx.enter_context(tc.tile_pool(name="ypool", bufs=5))
        tmp = ctx.enter_context(tc.tile_pool(name="tmp", bufs=3))
        sqp = ctx.enter_context(tc.tile_pool(name="sqp", bufs=2))
        stat = ctx.enter_context(tc.tile_pool(name="stat", bufs=4))
        statw = ctx.enter_context(tc.tile_pool(name="statw", bufs=3))
        grow = ctx.enter_context(tc.tile_pool(name="grow", bufs=3))
        biasp = ctx.enter_context(tc.tile_pool(name="biasp", bufs=3))
        wq_p = ctx.enter_context(tc.tile_pool(name="wq", bufs=4))
        wo_p = ctx.enter_context(tc.tile_pool(name="wop", bufs=5))
        wf1_p = ctx.enter_context(tc.tile_pool(name="wf1p", bufs=4))
        wf2_p = ctx.enter_context(tc.tile_pool(name="wf2p", bufs=6))
        qkvs = ctx.enter_context(tc.tile_pool(name="qkvs", bufs=3))
        attqk = ctx.enter_context(tc.tile_pool(name="attqk", bufs=1))
        attv = ctx.enter_context(tc.tile_pool(name="attv", bufs=17))
        ptp = ctx.enter_context(tc.tile_pool(name="ptp", bufs=3))
        atto = ctx.enter_context(tc.tile_pool(name="atto", bufs=3))
        afs = ctx.enter_context(tc.tile_pool(name="afs", bufs=2))
        afp = ctx.enter_context(tc.tile_pool(name="afp", bufs=5))
        gelup = ctx.enter_context(tc.tile_pool(name="gelup", bufs=3))
        outp = ctx.enter_context(tc.tile_pool(name="outp", bufs=2))
        ps_mm = ctx.enter_context(tc.tile_pool(name="psmm", bufs=2, space="PSUM"))
        ps_lg = ctx.enter_context(tc.tile_pool(name="pslg", bufs=2, space="PSUM"))
        ps_av = ctx.enter_context(tc.tile_pool(name="psav", bufs=2, space="PSUM"))

        # ---- internal DRAM for collectives ----------------------------
        a2a1_in = dram.tile([NCORES, 3, HD, TOK], BF16, tag="a2a1i")
        a2a1_out = dram.tile([NCORES, 3, HD, TOK], BF16, tag="a2a1o")
        a2a2_in = dram.tile([NCORES, HD, TOK], BF16, tag="a2a2i")
        a2a2_out = dram.tile([NCORES, HD, TOK], BF16, tag="a2a2o")

        # ---- constants -------------------------------------------------
        ones_f32 = cn.tile([128, 1], F32, tag="ones_f32")
        nc.vector.memset(ones_f32[:], 1.0)
        ones_row_f = cn.tile([1, 128], F32, tag="ones_row_f")
        nc.vector.memset(ones_row_f[:], 1.0)
        ones_w = cn.tile([1, 128], WDT, tag="ones_w")
        nc.vector.tensor_copy(ones_w[:], ones_row_f[:])
        eps_sb = cn.tile([1, 1], F32, tag="eps_sb")
        nc.vector.memset(eps_sb[:], EPS)
        trimask = cn.tile([128, 128], BF16, tag="trimask")
        nc.sync.dma_start(trimask[:], t["trimask_d"][:])
        win_sb = cn.tile([IN_DIM, D], WDT, tag="win_sb")
        nc.sync.dma_start(win_sb[:], t["win"][:])
        xT_sb = cn.tile([IN_DIM, TOK], WDT, tag="xT_sb")
        nc.sync.dma_start(xT_sb[:], t["xT"][:])
        binc_sb = cn.tile([128, DT], F32, tag="binc_sb")
        nc.sync.dma_start(binc_sb[:], t["binc"].rearrange("(a p) o -> p (a o)", p=128))
        wout_sb = [cn.tile([128, OUT], WDT, tag=f"wout{d}", name=f"wout_sb{d}") for d in range(DT)]
        for d in range(DT):
            nc.sync.dma_start(wout_sb[d][:], t["wout"][d * 128:(d + 1) * 128, :])
        boutr_sb = cn.tile([1, OUT], WDT, tag="boutr_sb")
        nc.sync.dma_start(boutr_sb[:], t["boutr"][:])
        lnfg_sb = cn.tile([1, D], WDT, tag="lnfg_sb")
        nc.sync.dma_start(lnfg_sb[:], t["lnfg"][:])
        lnfb_sb = cn.tile([128, DT], F32, tag="lnfb_sb")
        nc.sync.dma_start(lnfb_sb[:], t["lnfb"].rearrange("(a p) o -> p (a o)", p=128))

        # ---- residual stream (feature-major: h^T, 4 tiles [128, TOK]) --
        h = [hp.tile([128, TOK], F32, tag=f"h{d}", name=f"h{d}") for d in range(DT)]

        # ---- input projection: h = (x @ Win + bin)^T + pe^T ------------
        for d in range(DT):
            ps = ps_mm.tile([128, TOK], F32, tag="mm")
            nc.tensor.matmul(ps[:], win_sb[:, d * 128:(d + 1) * 128], xT_sb[:],
                             start=True, stop=True)
            pe_d = tmp.tile([128, TOK], F32, tag="tmp")
            nc.sync.dma_start(pe_d[:], t["peT"][d * 128:(d + 1) * 128, :])
            nc.vector.scalar_tensor_tensor(
                out=h[d][:], in0=ps[:], scalar=binc_sb[:, d:d + 1], in1=pe_d[:],
                op0=OP.add, op1=OP.add)

        def ln_emit(gr_tile, lnb_sb, bcol_idx=None):
            """Feature-major LN of h -> 4 WDT y tiles.
            y_d = (h_d - mu)*rstd*g_d + b_d, via:
              bcA = bcast(g_d * rstd), bcB = bcast(g_d * mu * rstd)
              y_d = (h_d * bcA + b_d) - bcB
            """
            sum_ps = ps_mm.tile([1, TOK], F32, tag="mm")
            sumsq_ps = ps_mm.tile([1, TOK], F32, tag="mm")
            for d in range(DT):
                sq = sqp.tile([128, TOK], F32, tag="sq")
                nc.vector.tensor_mul(sq[:], h[d][:], h[d][:])
                nc.tensor.matmul(sum_ps[:], ones_f32[:], h[d][:],
                                 start=(d == 0), stop=(d == DT - 1))
                nc.tensor.matmul(sumsq_ps[:], ones_f32[:], sq[:],
                                 start=(d == 0), stop=(d == DT - 1))
            mu = stat.tile([1, TOK], F32, tag="stat")
            nc.vector.tensor_scalar_mul(mu[:], sum_ps[:], 1.0 / D)
            var = stat.tile([1, TOK], F32, tag="stat")
            nc.vector.tensor_scalar_mul(var[:], sumsq_ps[:], 1.0 / D)
            mu2 = stat.tile([1, TOK], F32, tag="stat")
            nc.vector.tensor_mul(mu2[:], mu[:], mu[:])
            nc.vector.tensor_sub(var[:], var[:], mu2[:])
            nc.scalar.activation(var[:], var[:], AF.Ln, bias=eps_sb[:])
            rstd = stat.tile([1, TOK], F32, tag="stat")
            nc.scalar.activation(rstd[:], var[:], AF.Exp, scale=-0.5)
            rstd_w = statw.tile([1, TOK], WDT, tag="statw")
            nc.vector.tensor_copy(rstd_w[:], rstd[:])
            murstd_w = statw.tile([1, TOK], WDT, tag="statw")
            nc.vector.tensor_mul(murstd_w[:], mu[:], rstd[:])
            y = []
            for d in range(DT):
                gsl = gr_tile[:, d * 128:(d + 1) * 128]
                bcA = ps_mm.tile([128, TOK], F32, tag="mm")
                nc.tensor.matmul(bcA[:], gsl, rstd_w[:], start=True, stop=True)
                bcB = ps_mm.tile([128, TOK], F32, tag="mm")
                nc.tensor.matmul(bcB[:], gsl, murstd_w[:], start=True, stop=True)
                tt = tmp.tile([128, TOK], F32, tag="tmp")
                nc.vector.tensor_mul(tt[:], h[d][:], bcA[:])
                y_d = yp.tile([128, TOK], WDT, tag="y")
                nc.vector.scalar_tensor_tensor(
                    out=y_d[:], in0=tt[:], scalar=lnb_sb[:, d:d + 1], in1=bcB[:],
                    op0=OP.add, op1=OP.subtract)
                y.append(y_d)
            return y

        # =================== transformer layers ========================
        for l in range(L):
            # ---- LN1 --------------------------------------------------
            gr1 = grow.tile([1, D], WDT, tag="grow")
            nc.sync.dma_start(gr1[:], t["ln1g"][l])
            ln1b_sb = biasp.tile([128, DT], F32, tag="lnb")
            nc.sync.dma_start(ln1b_sb[:],
                              t["ln1b"][l].rearrange("(a p) o -> p (a o)", p=128))
            y1 = ln_emit(gr1, ln1b_sb)

            # ---- QKV: 12 chunks of 128 features -----------------------
            wq_sl = [wq_p.tile([128, 3 * D], WDT, tag="wq", name="wq_sl") for _ in range(DT)]
            for dtile in range(DT):
                nc.sync.dma_start(wq_sl[dtile][:],
                                  t["wqkv"][l, dtile * 128:(dtile + 1) * 128, :])
            bqkv_sb = biasp.tile([128, 12], F32, tag="bqkv")
            nc.sync.dma_start(bqkv_sb[:],
                              t["bqkvc"][l].rearrange("(a p) o -> p (a o)", p=128))
            for ch in range(12):
                ps = ps_mm.tile([128, TOK], F32, tag="mm")
                for dtile in range(DT):
                    nc.tensor.matmul(ps[:],
                                     wq_sl[dtile][:, ch * 128:(ch + 1) * 128],
                                     y1[dtile][:],
                                     start=(dtile == 0), stop=(dtile == DT - 1))
                stg = qkvs.tile([128, TOK], BF16, tag="qkvs")
                nc.vector.tensor_scalar_add(stg[:], ps[:], bqkv_sb[:, ch:ch + 1])
                kind, pr = ch // 4, ch % 4
                nc.sync.dma_start(a2a1_in[2 * pr, kind], stg[0:64, :])
                nc.sync.dma_start(a2a1_in[2 * pr + 1, kind], stg[64:128, :])

            # ---- AllToAll #1: per-head QKV to head owners --------------
            nc.gpsimd.collective_compute(
                "AllToAll", OP.bypass,
                replica_groups=[list(range(NCORES))],
                ins=[a2a1_in.opt()], outs=[a2a1_out.opt()])

            # ---- attention: my head, both batches ----------------------
            for b in range(B):
                qT = attqk.tile([HD, T], BF16, tag="qT")
                kT = attqk.tile([HD, T], BF16, tag="kT")
                for s in range(NSEG):
                    nc.sync.dma_start(qT[:, s * TOK:(s + 1) * TOK],
                                      a2a1_out[NSEG * b + s, 0])
                    nc.sync.dma_start(kT[:, s * TOK:(s + 1) * TOK],
                                      a2a1_out[NSEG * b + s, 1])
                vext = []
                for kc in range(NKC):
                    v = attv.tile([128, HD + 1], BF16, tag="vext")
                    nc.sync.dma_start_transpose(
                        v[:, 0:HD],
                        a2a1_out[NSEG * b + kc // 4, 2, :,
                                 (kc % 4) * 128:(kc % 4 + 1) * 128])
                    nc.vector.memset(v[:, HD:HD + 1], 1.0)
                    vext.append(v)
                for s in range(NSEG):
                    av = ps_av.tile([HD + 1, TOK], F32, tag="av")
                    nkc = 4 * s + 4
                    for kc0 in range(0, nkc, 2):
                        lg = ps_lg.tile([128, 1024], F32, tag="lg")
                        pt = ptp.tile([128, 1024], BF16, tag="pt")
                        offs = []
                        for half, kc in enumerate((kc0, kc0 + 1)):
                            off = max(0, (kc - 4 * s)) * 128
                            offs.append(off)
                            nc.tensor.matmul(
                                lg[:, half * 512 + off:(half + 1) * 512],
                                kT[:, kc * 128:(kc + 1) * 128],
                                qT[:, s * TOK + off:(s + 1) * TOK],
                                start=True, stop=True)
                        # exp over the needed ranges (PSUM -> SBUF bf16)
                        if offs[1] == 0:
                            nc.scalar.activation(pt[:, offs[0]:1024],
                                                 lg[:, offs[0]:1024],
                                                 AF.Exp, scale=SCALE)
                        else:
                            nc.scalar.activation(pt[:, offs[0]:512],
                                                 lg[:, offs[0]:512],
                                                 AF.Exp, scale=SCALE)
                            nc.scalar.activation(pt[:, 512 + offs[1]:1024],
                                                 lg[:, 512 + offs[1]:1024],
                                                 AF.Exp, scale=SCALE)
                        for half, kc in enumerate((kc0, kc0 + 1)):
                            if 4 * s <= kc:  # diagonal chunk: triangular mask
                                o = half * 512 + offs[half]
                                nc.vector.tensor_mul(pt[:, o:o + 128],
                                                     pt[:, o:o + 128],
                                                     trimask[:])
                            nc.tensor.matmul(
                                av[:, offs[half]:TOK],
                                vext[kc][:],
                                pt[:, half * 512 + offs[half]:(half + 1) * 512],
                                start=(kc == 0), stop=(kc == nkc - 1))
                    # normalize + ship to token owners
                    r = stat.tile([1, TOK], F32, tag="stat")
                    nc.vector.reciprocal(r[:], av[HD:HD + 1, :])
                    rw = statw.tile([1, TOK], WDT, tag="statw")
                    nc.vector.tensor_copy(rw[:], r[:])
                    rb = ps_mm.tile([HD, TOK], F32, tag="mm")
                    nc.tensor.matmul(rb[:], ones_w[0:1, 0:HD], rw[:],
                                     start=True, stop=True)
                    rb_sb = atto.tile([HD, TOK], F32, tag="rbsb")
                    nc.vector.tensor_copy(rb_sb[:], rb[:])
                    ao = atto.tile([HD, TOK], BF16, tag="ao")
                    nc.vector.tensor_tensor(ao[:], av[0:HD, :], rb_sb[:], OP.mult)
                    nc.sync.dma_start(a2a2_in[NSEG * b + s], ao[:])

            # ---- AllToAll #2: attention outputs back to token owners ---
            nc.gpsimd.collective_compute(
                "AllToAll", OP.bypass,
                replica_groups=[list(range(NCORES))],
                ins=[a2a2_in.opt()], outs=[a2a2_out.opt()])

            # ---- assemble attn^T [D, TOK] and convert to WDT -----------
            af = []
            for ft in range(DT):
                stg = afs.tile([128, TOK], BF16, tag="afs")
                nc.sync.dma_start(stg[0:64, :], a2a2_out[2 * ft])
                nc.sync.dma_start(stg[64:128, :], a2a2_out[2 * ft + 1])
                af_d = afp.tile([128, TOK], WDT, tag="af")
                nc.vector.tensor_copy(af_d[:], stg[:])
                af.append(af_d)

            # ---- out_proj + residual ----------------------------------
            wo_sl = [wo_p.tile([128, D], WDT, tag="wo", name="wo_sl") for _ in range(DT)]
            for ftile in range(DT):
                nc.sync.dma_start(wo_sl[ftile][:],
                                  t["wo"][l, ftile * 128:(ftile + 1) * 128, :])
            bo_sb = biasp.tile([128, DT], F32, tag="bo")
            nc.sync.dma_start(bo_sb[:],
                              t["boc"][l].rearrange("(a p) o -> p (a o)", p=128))
            for d in range(DT):
                ps = ps_mm.tile([128, TOK], F32, tag="mm")
                for ftile in range(DT):
                    nc.tensor.matmul(ps[:],
                                     wo_sl[ftile][:, d * 128:(d + 1) * 128],
                                     af[ftile][:],
                                     start=(ftile == 0), stop=(ftile == DT - 1))
                nc.vector.scalar_tensor_tensor(
                    out=h[d][:], in0=ps[:], scalar=bo_sb[:, d:d + 1], in1=h[d][:],
                    op0=OP.add, op1=OP.add)

            # ---- LN2 --------------------------------------------------
            gr2 = grow.tile([1, D], WDT, tag="grow")
            nc.sync.dma_start(gr2[:], t["ln2g"][l])
            ln2b_sb = biasp.tile([128, DT], F32, tag="lnb")
            nc.sync.dma_start(ln2b_sb[:],
                              t["ln2b"][l].rearrange("(a p) o -> p (a o)", p=128))
            y2 = ln_emit(gr2, ln2b_sb)

            # ---- fc1 + gelu -------------------------------------------
            wf1_sl = [wf1_p.tile([128, FF], WDT, tag="wf1", name="wf1_sl") for _ in range(DT)]
            for dtile in range(DT):
                nc.sync.dma_start(wf1_sl[dtile][:],
                                  t["wf1"][l, dtile * 128:(dtile + 1) * 128, :])
            bf1_sb = biasp.tile([128, FFT], F32, tag="bf1")
            nc.sync.dma_start(bf1_sb[:],
                              t["bf1c"][l].rearrange("(a p) o -> p (a o)", p=128))
            gl = []
            for ch in range(FFT):
                ps = ps_mm.tile([128, TOK], F32, tag="mm")
                for dtile in range(DT):
                    nc.tensor.matmul(ps[:],
                                     wf1_sl[dtile][:, ch * 128:(ch + 1) * 128],
                                     y2[dtile][:],
                                     start=(dtile == 0), stop=(dtile == DT - 1))
                g_t = gelup.tile([128, TOK], WDT, tag="gelu")
                nc.scalar.activation(g_t[:], ps[:], AF.Gelu,
                                     bias=bf1_sb[:, ch:ch + 1])
                gl.append(g_t)

            # ---- fc2 + residual (ff-outer, 4 psum accumulators) --------
            bf2_sb = biasp.tile([128, DT], F32, tag="bo")
            nc.sync.dma_start(bf2_sb[:],
                              t["bf2c"][l].rearrange("(a p) o -> p (a o)", p=128))
            acc_lg = [ps_lg.tile([128, 1024], F32, tag="lg", name="acc_lg") for _ in range(2)]
            accs = [acc_lg[0][:, 0:512], acc_lg[0][:, 512:1024],
                    acc_lg[1][:, 0:512], acc_lg[1][:, 512:1024]]
            for fft in range(FFT):
                sl = wf2_p.tile([128, D], WDT, tag="wf2")
                nc.sync.dma_start(sl[:], t["wf2"][l, fft * 128:(fft + 1) * 128, :])
                for d in range(DT):
                    nc.tensor.matmul(accs[d],
                                     sl[:, d * 128:(d + 1) * 128],
                                     gl[fft][:],
                                     start=(fft == 0), stop=(fft == FFT - 1))
            for d in range(DT):
                nc.vector.scalar_tensor_tensor(
                    out=h[d][:], in0=accs[d], scalar=bf2_sb[:, d:d + 1],
                    in1=h[d][:], op0=OP.add, op1=OP.add)

        # ---- final LN + output head -----------------------------------
        yf = ln_emit(lnfg_sb, lnfb_sb)
        for tc_ in range(DT):
            ps = ps_mm.tile([128, OUT], F32, tag="mm")
            for dtile in range(DT):
                nc.tensor.matmul(ps[:],
                                 yf[dtile][:, tc_ * 128:(tc_ + 1) * 128],
                                 wout_sb[dtile][:],
                                 start=(dtile == 0), stop=False)
            nc.tensor.matmul(ps[:], ones_w[:], boutr_sb[:],
                             start=False, stop=True)
            o_sb = outp.tile([128, OUT], F32, tag="osb")
            nc.vector.tensor_copy(o_sb[:], ps[:])
            nc.sync.dma_start(t["out_d"][tc_ * 128:(tc_ + 1) * 128, :], o_sb[:])


_NC_CACHE = None


def _get_program():
    global _NC_CACHE
    if _NC_CACHE is None:
        _NC_CACHE = _build_program()
    return _NC_CACHE


def _pe_table():
    pos = np.arange(T, dtype=np.float32)[:, None]
    div = np.exp(np.arange(0, D, 2, dtype=np.float32)
                 * np.float32(-math.log(10000.0) / D))
    pe = np.zeros((T, D), np.float32)
    pe[:, 0::2] = np.sin(pos * div)
    pe[:, 1::2] = np.cos(pos * div)
    return pe


def _make_in_maps(inputs):
    x = np.asarray(inputs["x"], np.float32)
    pe = _pe_table()
    tri = np.triu(np.ones((128, 128), np.float32)).astype(ml_dtypes.bfloat16)

    def w(a):
        return np.ascontiguousarray(np.asarray(a, np.float32)).astype(WNP)

    shared = {
        "win": w(inputs["Win"]),
        "binc": np.ascontiguousarray(np.asarray(inputs["bin_"], np.float32)
                                     .reshape(D, 1)),
        "wqkv": w(inputs["Wqkv"]),
        "bqkvc": np.asarray(inputs["bqkv"], np.float32).reshape(L, 3 * D, 1),
        "wo": w(inputs["Wo"]),
        "boc": np.asarray(inputs["bo"], np.float32).reshape(L, D, 1),
        "wf1": w(inputs["Wf1"]),
        "bf1c": np.asarray(inputs["bf1"], np.float32).reshape(L, FF, 1),
        "wf2": w(inputs["Wf2"]),
        "bf2c": np.asarray(inputs["bf2"], np.float32).reshape(L, D, 1),
        "ln1g": w(np.asarray(inputs["ln1_g"]).reshape(L, 1, D)),
        "ln1b": np.asarray(inputs["ln1_b"], np.float32).reshape(L, D, 1),
        "ln2g": w(np.asarray(inputs["ln2_g"]).reshape(L, 1, D)),
        "ln2b": np.asarray(inputs["ln2_b"], np.float32).reshape(L, D, 1),
        "lnfg": w(np.asarray(inputs["lnf_g"]).reshape(1, D)),
        "lnfb": np.asarray(inputs["lnf_b"], np.float32).reshape(D, 1),
        "wout": w(inputs["Wout"]),
        "boutr": w(np.asarray(inputs["bout"]).reshape(1, OUT)),
        "trimask": tri,
    }
    in_maps = []
    for c in range(NCORES):
        bb, j = c // 4, c % 4
        m = dict(shared)
        m["xT"] = np.ascontiguousarray(
            x[bb, j * TOK:(j + 1) * TOK, :].T).astype(WNP)
        m["peT"] = np.ascontiguousarray(pe[j * TOK:(j + 1) * TOK, :].T)
        in_maps.append(m)
    return in_maps


def kernel(**inputs):
    nc = _get_program()
    in_maps = _make_in_maps(inputs)
    res = run_bass_kernel_spmd(nc, in_maps, list(range(NCORES)))
    out = np.zeros((B, T, OUT), np.float32)
    for c in range(NCORES):
        bb, j = c // 4, c % 4
        out[bb, j * TOK:(j + 1) * TOK, :] = res.results[c]["out"]
    return out
